# revision 43
# baseline (speedup 1.0000x reference)
"""Trainium2 Bass kernel for the PINN-style loss problem.

Math: a 6-layer tanh MLP u(x,t) (2->50x5->1) is evaluated with forward-mode
jets (u, u_x, u_t, u_xxx) at N=10000 points. The per-param loss
  loss_p = mean_n (u_t + a_p*u*u_x + b_p*u_xxx + c_p*u_x)^2
collapses to the quadratic form loss_p = q4^T G q4 / N with q4 = [a,b,c,1]
and G the 4x4 Gram of g_n = [u*u_x, u_xxx, u_x, u_t].

Sharding: x is split into 8 slices of 1250 points (one per NeuronCore);
each core builds its partial Gram (pre-scaled by 1/N), the 8 Grams are
exchanged with peer-to-peer remote DMA broadcasts (no collectives), and
each core evaluates the quadratic form for its 625-row slice of para.

Device layout: points are packed 2-per-partition-block (block-diagonal
weights, K=100), free dim 640 per block (block0: 640 real points,
block1: 610 real + 30 zero-padded, masked out before the Gram matmul).
"""

import os
import sys
import numpy as np

for _p in ("/opt/trn_rl_repo",):
    if os.path.isdir(_p) and _p not in sys.path:
        sys.path.append(_p)

import concourse.bass as bass
import concourse.bacc as bacc
import concourse.mybir as mybir
import concourse.tile as tile
from concourse import bass_utils

F32 = mybir.dt.float32
F32R = mybir.dt.float32r
F16 = mybir.dt.float16
U32 = mybir.dt.uint32
AF = mybir.ActivationFunctionType
ALU = mybir.AluOpType

NCORES = 8
NPTS = 10000
NPC = NPTS // NCORES       # 1250 points per core
PPC = 5000 // NCORES       # 625 para rows per core
FD = 640                   # free dim per block (block0 full, block1 padded)
B1 = NPC - FD              # 610 real points in block1
HB = 100                   # 2 blocks x 50 hidden units
CHUNKS = ((0, 512), (512, 128))      # matmul free-dim chunks (psum bank limit)
PPCP = 640                           # para rows padded (f32r needs N % 4 == 0)
PCH = ((0, 512), (512, PPCP - 512))  # para free-dim chunks

TOWER_F16 = False          # fp16 tower: DVE/GpSimd run 2-byte ops ~2x SLOWER
SDT = F16 if TOWER_F16 else F32R   # stream/weight dtype (1 cyc/row matmul)
IDT = F16 if TOWER_F16 else F32    # elementwise intermediate dtype
NDT = np.float16 if TOWER_F16 else np.float32  # host-side stream dtype
USE_RDMA = False           # peer-to-peer Gram exchange instead of AllReduce
RDMA_BARRIER = False       # bir_kernel_barrier before trigger (adds CC prelude)
WARM_CC = False            # early dummy collective (AllReduce path only)


def _mm(nc, out, lhsT, rhs, start=True, stop=True):
    nc.tensor.matmul(out, lhsT, rhs, start=start, stop=stop)


def _mm_chunks(nc, out_tile, lhsT, rhs_tile, chunks=CHUNKS):
    for off, w in chunks:
        _mm(nc, out_tile[:, off:off + w], lhsT, rhs_tile[:, off:off + w])


def build_program(stage="full"):
    nc = bacc.Bacc("TRN2", target_bir_lowering=False, debug=False)

    h0_d = nc.dram_tensor("h0", [4, FD], SDT, kind="ExternalInput")
    paraT_d = nc.dram_tensor("paraT", [4, PPCP], F32R, kind="ExternalInput")
    w1t_d = nc.dram_tensor("w1t", [4, HB], SDT, kind="ExternalInput")
    wb_d = nc.dram_tensor("wb", [HB, 400], SDT, kind="ExternalInput")
    w6p_d = nc.dram_tensor("w6p", [HB, 2], SDT, kind="ExternalInput")
    vecs_d = nc.dram_tensor("vecs", [HB, 10], F32, kind="ExternalInput")
    b6bc_d = nc.dram_tensor("b6bc", [128, 2], F32, kind="ExternalInput")
    cid_d = nc.dram_tensor("cid", [1, 1], U32, kind="ExternalInput")
    if stage == "tower":
        loss_d = nc.dram_tensor("dbg", [HB, FD], SDT, kind="ExternalOutput")
    elif stage in ("l6", "cc"):
        loss_d = nc.dram_tensor("dbg", [4, 4], F32, kind="ExternalOutput")
    else:  # full
        loss_d = nc.dram_tensor("loss", [1, PPC], F32, kind="ExternalOutput")

    with tile.TileContext(nc) as tc:
        _body(tc, nc, h0_d, paraT_d, w1t_d, wb_d, w6p_d, vecs_d, b6bc_d,
              cid_d, loss_d, stage=stage)
    nc.compile()
    return nc


def _body(tc, nc, h0_d, paraT_d, w1t_d, wb_d, w6p_d, vecs_d, b6bc_d, cid_d,
          loss_d, stage="full"):
    import contextlib

    ctx = contextlib.ExitStack()
    with ctx:
        cpool = ctx.enter_context(tc.tile_pool(name="const", bufs=1))
        spool = ctx.enter_context(tc.tile_pool(name="streams", bufs=2))
        tpool = ctx.enter_context(tc.tile_pool(name="trans", bufs=2))
        dpool = ctx.enter_context(tc.tile_pool(name="dram", bufs=1, space="DRAM"))

        v = nc.vector
        s = nc.scalar
        g = nc.gpsimd

        # ---- load constants (spread across engine DMA queues) ----
        h0 = cpool.tile([4, FD], SDT, tag="h0")
        w1t = cpool.tile([4, HB], SDT, tag="w1t")
        vecs = cpool.tile([HB, 10], F32, tag="vecs")
        wb = cpool.tile([HB, 400], SDT, tag="wb")
        w6p = cpool.tile([HB, 2], SDT, tag="w6p")
        b6bc = cpool.tile([128, 2], F32, tag="b6bc")
        q4 = cpool.tile([4, PPCP], F32R, tag="q4")
        cid_s = cpool.tile([1, 1], U32, tag="cid")
        nc.sync.dma_start(h0[:], h0_d[:])
        s.dma_start(w1t[:], w1t_d[:])
        s.dma_start(vecs[:], vecs_d[:])
        nc.sync.dma_start(wb[:], wb_d[:])
        s.dma_start(w6p[:], w6p_d[:])
        s.dma_start(b6bc[:], b6bc_d[:])
        g.dma_start(q4[:], paraT_d[:])
        g.dma_start(cid_s[:], cid_d[:])

        ones4 = cpool.tile([4, 1], F32, tag="ones4")
        v.memset(ones4[:], 1.0)
        gS = cpool.tile([128, 4], F32, tag="gS")
        recv = cpool.tile([128, 4 * NCORES], F32, tag="recv")
        g.memset(gS[:], 0.0)
        g.memset(recv[:], 0.0)

        # p-state warmup: engines ramp to full clock only after a few us of
        # continuous work; burn the DMA-wait window on dummy ops so layer 1
        # doesn't pay the ~8-10us cold-engine penalty.
        vscr = cpool.tile([128, 512], F32, tag="vscr")
        gscr = cpool.tile([128, 512], F32, tag="gscr")
        for _ in range(10):
            v.memset(vscr[:], 0.0)
        for _ in range(5):
            g.memset(gscr[:], 0.0)

        if USE_RDMA:
            cid_val = nc.values_load(
                cid_s[0:1, 0:1], engines=[mybir.EngineType.Pool],
                min_val=0, max_val=NCORES - 1, skip_runtime_bounds_check=True)
            bsem = nc.alloc_semaphore("gex_arrive")
            lsem = nc.alloc_semaphore("gex_local")
            psem = nc.alloc_semaphore("gex_prep")

        if (not USE_RDMA) and WARM_CC:
            win = dpool.tile([1, 1], F32, tag="win")
            wout = dpool.tile([1, 1], F32, tag="wout")
            g.dma_start(win[:], ones4[0:1, 0:1])
            g.collective_compute(
                "AllReduce", ALU.add,
                replica_groups=[list(range(NCORES))],
                ins=[win.opt()], outs=[wout.opt()],
            )

        cx = vecs[:, 0:1]
        ct = vecs[:, 1:2]
        cx2 = vecs[:, 2:3]
        cx3 = vecs[:, 3:4]

        def bb(layer):  # bias vector for layer 1..5
            return vecs[:, 3 + layer:4 + layer]

        neg2 = vecs[:, 9:10]

        a5 = ax5 = at5 = axxx5 = None

        with tc.tile_pool(name="ztw", bufs=4, space="PSUM") as zpool:
            # ---------- layer 1 ----------
            # zx/zt/zxx/zxxx for layer1 are per-unit constants cx/ct/etc.
            z = zpool.tile([HB, FD], F32, tag="ztw")
            _mm_chunks(nc, z, w1t[:], h0)
            a = spool.tile([HB, FD], SDT, tag="a")
            s.activation(a[:], z[:], AF.Tanh, bias=bb(1))
            asq = tpool.tile([HB, FD], IDT, tag="asq")
            s.activation(asq[:], a[:], AF.Square)
            h6 = tpool.tile([HB, FD], IDT, tag="h6")
            s.activation(h6[:], asq[:], AF.Identity, scale=6.0, bias=neg2)
            f1 = tpool.tile([HB, FD], IDT, tag="f1")
            s.activation(f1[:], asq[:], AF.Identity, scale=-1.0, bias=1.0)
            ax = spool.tile([HB, FD], SDT, tag="ax")
            v.tensor_scalar(ax[:], f1[:], cx, None, ALU.mult)
            at = spool.tile([HB, FD], SDT, tag="at")
            g.tensor_scalar(at[:], f1[:], ct, None, ALU.mult)
            af1 = tpool.tile([HB, FD], IDT, tag="t2")
            v.tensor_tensor(af1[:], a[:], f1[:], ALU.mult)
            axx = spool.tile([HB, FD], SDT, tag="axx")
            v.tensor_scalar(axx[:], af1[:], cx2, -2.0, ALU.mult, ALU.mult)
            fh = tpool.tile([HB, FD], IDT, tag="t4")
            g.tensor_tensor(fh[:], f1[:], h6[:], ALU.mult)
            axxx = spool.tile([HB, FD], SDT, tag="axxx")
            v.tensor_scalar(axxx[:], fh[:], cx3, None, ALU.mult)

            # ---------- layers 2..5 ----------
            # y=tanh(Wx+b); with f1=1-a^2, h6=6a^2-2:
            #   at   = f1*zt
            #   ax   = f1*zx
            #   axx  = f1*(zxx - 2a*zx^2)
            #   axxx = f1*zxxx + ax*(h6*zx^2 - 6a*zxx)
            for layer in range(2, 6):
                W = wb[:, 100 * (layer - 2):100 * (layer - 1)]
                last = layer == 5

                z = zpool.tile([HB, FD], F32, tag="ztw")
                _mm_chunks(nc, z, W, a)
                zx = zpool.tile([HB, FD], F32, tag="ztw")
                _mm_chunks(nc, zx, W, ax)
                a_n = spool.tile([HB, FD], SDT, tag="a")
                s.activation(a_n[:], z[:], AF.Tanh, bias=bb(layer))
                asq = tpool.tile([HB, FD], IDT, tag="asq")
                s.activation(asq[:], a_n[:], AF.Square)
                w2 = tpool.tile([HB, FD], IDT, tag="w2")
                s.activation(w2[:], zx[:], AF.Square)

                zxx = zpool.tile([HB, FD], F32, tag="ztw")
                _mm_chunks(nc, zxx, W, axx)
                h6 = tpool.tile([HB, FD], IDT, tag="h6")
                s.activation(h6[:], asq[:], AF.Identity, scale=6.0, bias=neg2)
                f1 = tpool.tile([HB, FD], IDT, tag="f1")
                s.activation(f1[:], asq[:], AF.Identity, scale=-1.0, bias=1.0)
                ax_n = spool.tile([HB, FD], SDT, tag="ax")
                v.tensor_tensor(ax_n[:], f1[:], zx[:], ALU.mult)
                t2 = tpool.tile([HB, FD], IDT, tag="t2")
                v.tensor_tensor(t2[:], a_n[:], zxx[:], ALU.mult)
                t4 = tpool.tile([HB, FD], IDT, tag="t4")
                g.tensor_tensor(t4[:], h6[:], w2[:], ALU.mult)
                t5 = tpool.tile([HB, FD], IDT, tag="t5")
                v.scalar_tensor_tensor(t5[:], t2[:], -6.0, t4[:],
                                       ALU.mult, ALU.add)
                t6 = tpool.tile([HB, FD], IDT, tag="t6")
                v.tensor_tensor(t6[:], ax_n[:], t5[:], ALU.mult)

                zxxx = zpool.tile([HB, FD], F32, tag="ztw")
                _mm_chunks(nc, zxxx, W, axxx)
                t1 = tpool.tile([HB, FD], IDT, tag="t1")
                v.tensor_tensor(t1[:], f1[:], zxxx[:], ALU.mult)
                axxx_n = spool.tile([HB, FD], SDT, tag="axxx")
                v.tensor_tensor(axxx_n[:], t1[:], t6[:], ALU.add)

                zt = zpool.tile([HB, FD], F32, tag="ztw")
                _mm_chunks(nc, zt, W, at)
                at_n = spool.tile([HB, FD], SDT, tag="at")
                v.tensor_tensor(at_n[:], f1[:], zt[:], ALU.mult)

                if not last:
                    gt = tpool.tile([HB, FD], IDT, tag="gt")
                    g.tensor_tensor(gt[:], a_n[:], w2[:], ALU.mult)
                    inner = tpool.tile([HB, FD], IDT, tag="inner")
                    v.scalar_tensor_tensor(inner[:], gt[:], -2.0, zxx[:],
                                           ALU.mult, ALU.add)
                    axx_n = spool.tile([HB, FD], SDT, tag="axx")
                    g.tensor_tensor(axx_n[:], f1[:], inner[:], ALU.mult)
                    axx = axx_n

                a, at, ax, axxx = a_n, at_n, ax_n, axxx_n

            a5, ax5, at5, axxx5 = a, ax, at, axxx

        if stage == "tower":
            nc.sync.dma_start(loss_d[:], axxx5[:])
            return

        # ---------- layer 6 + Gram ----------
        # chunk tiles: [128 points, 10] cols: s-major pairs (b0,b1) for
        # s=0 uux, 1 uxxx, 2 ux, 3 ut; cols 8:10 = u.
        with tc.tile_pool(name="l6c", bufs=3, space="PSUM") as l6p, \
             tc.tile_pool(name="psmall", bufs=1, space="PSUM") as pps:
            G = pps.tile([4, 4], F32, tag="gram")
            chvs = []
            for c in range(5):
                lo = 128 * c
                ch = l6p.tile([128, 10], F32, tag="l6c")
                _mm(nc, ch[:, 8:10], a5[:, lo:lo + 128], w6p[:])
                _mm(nc, ch[:, 2:4], axxx5[:, lo:lo + 128], w6p[:])
                _mm(nc, ch[:, 4:6], ax5[:, lo:lo + 128], w6p[:])
                _mm(nc, ch[:, 6:8], at5[:, lo:lo + 128], w6p[:])
                chS = tpool.tile([128, 10], F32, tag=f"l6s{c}")
                s.activation(chS[:, 2:10], ch[:, 2:10], AF.Identity)
                # uux = (u + b6) * ux
                v.scalar_tensor_tensor(chS[:, 0:2], chS[:, 8:10], b6bc[:128, 0:1],
                                       chS[:, 4:6], ALU.add, ALU.mult)
                chv = chS[:, 0:8].rearrange("p (s b) -> p b s", b=2, s=4)
                if c == 4 and B1 < FD:
                    # zero the padded block1 points before the Gram matmul
                    v.tensor_scalar(chv[:, 1, :], chv[:, 1, :], b6bc[:128, 1:2],
                                    None, ALU.mult)
                chvs.append(chv)
            for c in range(5):
                for b in range(2):
                    st = c == 0 and b == 0
                    sp = c == 4 and b == 1
                    nc.tensor.matmul(G[:], chvs[c][:, b, :], chvs[c][:, b, :],
                                     start=st, stop=sp)

            # partial Gram -> SBUF, pre-scaled by 1/N
            s.activation(gS[0:4, :], G[:], AF.Identity, scale=1.0 / NPTS)

            if stage == "l6":
                nc.sync.dma_start(loss_d[:], gS[0:4, :])
                return

            # ---------- exchange partial Grams across the 8 cores ----------
            if USE_RDMA:
                # Each core broadcasts its [128,4] gS to all 8 peers
                # (including itself), into its own column block of recv.
                # All 8 broadcasts -> remote_sem += 2 each -> wait 16.
                for c in range(NCORES):
                    with tc.If(cid_val == c, preferred_fallthrough_block=False):
                        g.remote_dma_broadcast(
                            recv[:, 4 * c:4 * c + 4], gS[:, :],
                            remote_sem=bsem, local_sem=lsem,
                            rdests=[(0, k) for k in range(NCORES)],
                        ).then_inc(psem, 1)
                g.wait_ge(psem, 1)
                if RDMA_BARRIER:
                    g.bir_kernel_barrier_wait([list(range(NCORES))])
                g.trigger_dma(count=1)
                v.wait_ge(bsem, 16)
                Gsum = cpool.tile([4, 4], F32, tag="Gsum")
                rv = recv[0:4, :].rearrange("p (c j) -> p j c", c=NCORES, j=4)
                v.tensor_reduce(Gsum[:], rv, mybir.AxisListType.X, ALU.add)
            else:
                gin = dpool.tile([4, 4], F32, tag="gin")
                gout = dpool.tile([4, 4], F32, tag="gout")
                g.dma_start(gin[:], gS[0:4, :])
                g.collective_compute(
                    "AllReduce", ALU.add,
                    replica_groups=[list(range(NCORES))],
                    ins=[gin.opt()], outs=[gout.opt()],
                )
                Gsum = cpool.tile([4, 4], F32, tag="Gsum")
                g.dma_start(Gsum[:], gout[:])

            if stage == "cc":
                nc.sync.dma_start(loss_d[:], Gsum[:].bitcast(F32))
                return

            # ---------- para quadratic form ----------
            # loss = q4^T G q4 / N  (1/N already folded into G)
            Y = pps.tile([4, PPCP], F32, tag="Y")
            for off, w in PCH:
                _mm(nc, Y[:, off:off + w], Gsum[:].bitcast(F32R),
                    q4[:, off:off + w])
            tq = cpool.tile([4, PPCP], F32R, tag="tq")
            v.tensor_tensor(tq[:], Y[:], q4[:], ALU.mult)
            LP = pps.tile([1, PPCP], F32, tag="LP")
            for off, w in PCH:
                _mm(nc, LP[:, off:off + w], ones4[:].bitcast(F32R),
                    tq[:, off:off + w])
            lossS = cpool.tile([1, PPC], F32, tag="lossS")
            v.tensor_copy(lossS[:], LP[:, 0:PPC])
            nc.sync.dma_start(loss_d[:], lossS[:])


def prep_inputs(x, para, W1, b1, W2, b2, W3, b3, W4, b4, W5, b5, W6, b6):
    """Full inputs -> list of per-core input dicts (host-side shard/layout)."""
    f = np.float32
    x = np.asarray(x, f)
    para = np.asarray(para, f)
    Ws = [np.asarray(W, f) for W in (W1, W2, W3, W4, W5, W6)]
    bs = [np.asarray(b, f) for b in (b1, b2, b3, b4, b5, b6)]

    w1t = np.zeros((4, HB), NDT)
    w1t[0:2, 0:50] = Ws[0].T
    w1t[2:4, 50:100] = Ws[0].T
    wb = np.zeros((HB, 400), NDT)
    for i in range(4):
        W = Ws[i + 1]
        wb[0:50, 100 * i:100 * i + 50] = W.T
        wb[50:100, 100 * i + 50:100 * i + 100] = W.T
    w6p = np.zeros((HB, 2), NDT)
    w6p[0:50, 0] = Ws[5][0]
    w6p[50:100, 1] = Ws[5][0]
    vecs = np.zeros((HB, 10), f)
    vecs[:, 9] = -2.0
    cx = Ws[0][:, 0]
    ct = Ws[0][:, 1]
    for half in (slice(0, 50), slice(50, 100)):
        vecs[half, 0] = cx
        vecs[half, 1] = ct
        vecs[half, 2] = cx * cx
        vecs[half, 3] = cx * cx * cx
        for l in range(5):
            vecs[half, 4 + l] = bs[l]
    b6bc = np.zeros((128, 2), f)
    b6bc[:, 0] = bs[5][0]
    b6bc[:, 1] = 1.0
    b6bc[B1 - 512:, 1] = 0.0

    maps = []
    for c in range(NCORES):
        sl = x[c * NPC:(c + 1) * NPC]
        h0 = np.zeros((4, FD), NDT)
        h0[0] = sl[0:FD, 0]
        h0[1] = sl[0:FD, 1]
        h0[2, 0:B1] = sl[FD:NPC, 0]
        h0[3, 0:B1] = sl[FD:NPC, 1]
        paraT = np.ones((4, PPCP), f)
        paraT[0:3, 0:PPC] = para[c * PPC:(c + 1) * PPC].T
        maps.append({
            "h0": h0, "paraT": paraT, "w1t": w1t, "wb": wb,
            "w6p": w6p, "vecs": vecs, "b6bc": b6bc,
            "cid": np.array([[c]], np.uint32),
        })
    return maps


_NC_CACHE = {}


def get_program():
    if "nc" not in _NC_CACHE:
        _NC_CACHE["nc"] = build_program()
    return _NC_CACHE["nc"]


def kernel(x, para, W1, b1, W2, b2, W3, b3, W4, b4, W5, b5, W6, b6):
    maps = prep_inputs(x, para, W1, b1, W2, b2, W3, b3, W4, b4, W5, b5, W6, b6)
    nc = get_program()
    res = bass_utils.run_bass_kernel_spmd(nc, maps, list(range(NCORES)))
    out = np.concatenate([res.results[c]["loss"].reshape(-1) for c in range(NCORES)])
    return out.astype(np.float32)


# revision 45
# speedup vs baseline: 1.1935x; 1.1935x over previous
"""Trainium2 Bass kernel for the PINN-style loss problem.

Math: a 6-layer tanh MLP u(x,t) (2->50x5->1) is evaluated with forward-mode
jets (u, u_x, u_t, u_xxx) at N=10000 points. The per-param loss
  loss_p = mean_n (u_t + a_p*u*u_x + b_p*u_xxx + c_p*u_x)^2
collapses to the quadratic form loss_p = q4^T G q4 / N with q4 = [a,b,c,1]
and G the 4x4 Gram of g_n = [u*u_x, u_xxx, u_x, u_t].

Sharding: x is split into 8 slices of 1250 points (one per NeuronCore);
each core builds its partial Gram (pre-scaled by 1/N), the 8 Grams are
exchanged with peer-to-peer remote DMA broadcasts (no collectives), and
each core evaluates the quadratic form for its 625-row slice of para.

Device layout: points are packed 2-per-partition-block (block-diagonal
weights, K=100), free dim 640 per block (block0: 640 real points,
block1: 610 real + 30 zero-padded, masked out before the Gram matmul).
"""

import os
import sys
import numpy as np

for _p in ("/opt/trn_rl_repo",):
    if os.path.isdir(_p) and _p not in sys.path:
        sys.path.append(_p)

import concourse.bass as bass
import concourse.bacc as bacc
import concourse.mybir as mybir
import concourse.tile as tile
from concourse import bass_utils

F32 = mybir.dt.float32
F32R = mybir.dt.float32r
F16 = mybir.dt.float16
U32 = mybir.dt.uint32
AF = mybir.ActivationFunctionType
ALU = mybir.AluOpType

NCORES = 8
NPTS = 10000
NPC = NPTS // NCORES       # 1250 points per core
PPC = 5000 // NCORES       # 625 para rows per core
FD = 640                   # free dim per block (block0 full, block1 padded)
B1 = NPC - FD              # 610 real points in block1
HB = 100                   # 2 blocks x 50 hidden units
CHUNKS = ((0, 512), (512, 128))      # matmul free-dim chunks (psum bank limit)
PPCP = 640                           # para rows padded (f32r needs N % 4 == 0)
PCH = ((0, 512), (512, PPCP - 512))  # para free-dim chunks

TOWER_F16 = False          # fp16 tower: DVE/GpSimd run 2-byte ops ~2x SLOWER
SDT = F16 if TOWER_F16 else F32R   # stream/weight dtype (1 cyc/row matmul)
IDT = F16 if TOWER_F16 else F32    # elementwise intermediate dtype
NDT = np.float16 if TOWER_F16 else np.float32  # host-side stream dtype
USE_RDMA = False           # peer-to-peer Gram exchange instead of AllReduce
RDMA_BARRIER = False       # bir_kernel_barrier before trigger (adds CC prelude)
WARM_CC = True             # early dummy collective (AllReduce path only)


def _mm(nc, out, lhsT, rhs, start=True, stop=True):
    nc.tensor.matmul(out, lhsT, rhs, start=start, stop=stop)


def _mm_chunks(nc, out_tile, lhsT, rhs_tile, chunks=CHUNKS):
    for off, w in chunks:
        _mm(nc, out_tile[:, off:off + w], lhsT, rhs_tile[:, off:off + w])


def build_program(stage="full"):
    nc = bacc.Bacc("TRN2", target_bir_lowering=False, debug=False)

    h0_d = nc.dram_tensor("h0", [4, FD], SDT, kind="ExternalInput")
    paraT_d = nc.dram_tensor("paraT", [4, PPCP], F32R, kind="ExternalInput")
    w1t_d = nc.dram_tensor("w1t", [4, HB], SDT, kind="ExternalInput")
    wb_d = nc.dram_tensor("wb", [HB, 400], SDT, kind="ExternalInput")
    w6p_d = nc.dram_tensor("w6p", [HB, 2], SDT, kind="ExternalInput")
    vecs_d = nc.dram_tensor("vecs", [HB, 10], F32, kind="ExternalInput")
    b6bc_d = nc.dram_tensor("b6bc", [128, 2], F32, kind="ExternalInput")
    cid_d = nc.dram_tensor("cid", [1, 1], U32, kind="ExternalInput")
    if stage == "tower":
        loss_d = nc.dram_tensor("dbg", [HB, FD], SDT, kind="ExternalOutput")
    elif stage in ("l6", "cc"):
        loss_d = nc.dram_tensor("dbg", [4, 4], F32, kind="ExternalOutput")
    else:  # full
        loss_d = nc.dram_tensor("loss", [1, PPC], F32, kind="ExternalOutput")

    with tile.TileContext(nc) as tc:
        _body(tc, nc, h0_d, paraT_d, w1t_d, wb_d, w6p_d, vecs_d, b6bc_d,
              cid_d, loss_d, stage=stage)
    nc.compile()
    return nc


def _body(tc, nc, h0_d, paraT_d, w1t_d, wb_d, w6p_d, vecs_d, b6bc_d, cid_d,
          loss_d, stage="full"):
    import contextlib

    ctx = contextlib.ExitStack()
    with ctx:
        cpool = ctx.enter_context(tc.tile_pool(name="const", bufs=1))
        spool = ctx.enter_context(tc.tile_pool(name="streams", bufs=2))
        tpool = ctx.enter_context(tc.tile_pool(name="trans", bufs=2))
        dpool = ctx.enter_context(tc.tile_pool(name="dram", bufs=1, space="DRAM"))

        v = nc.vector
        s = nc.scalar
        g = nc.gpsimd

        # ---- load constants (spread across engine DMA queues) ----
        h0 = cpool.tile([4, FD], SDT, tag="h0")
        w1t = cpool.tile([4, HB], SDT, tag="w1t")
        vecs = cpool.tile([HB, 10], F32, tag="vecs")
        wb = cpool.tile([HB, 400], SDT, tag="wb")
        w6p = cpool.tile([HB, 2], SDT, tag="w6p")
        b6bc = cpool.tile([128, 2], F32, tag="b6bc")
        q4 = cpool.tile([4, PPCP], F32R, tag="q4")
        cid_s = cpool.tile([1, 1], U32, tag="cid")
        nc.sync.dma_start(h0[:], h0_d[:])
        s.dma_start(w1t[:], w1t_d[:])
        s.dma_start(vecs[:], vecs_d[:])
        nc.sync.dma_start(wb[:], wb_d[:])
        s.dma_start(w6p[:], w6p_d[:])
        s.dma_start(b6bc[:], b6bc_d[:])
        g.dma_start(q4[:], paraT_d[:])
        g.dma_start(cid_s[:], cid_d[:])

        ones4 = cpool.tile([4, 1], F32, tag="ones4")
        v.memset(ones4[:], 1.0)
        gS = cpool.tile([128, 4], F32, tag="gS")
        recv = cpool.tile([128, 4 * NCORES], F32, tag="recv")
        g.memset(gS[:], 0.0)
        g.memset(recv[:], 0.0)

        # Opcode warmup: the FIRST TensorTensor / TensorScalar / stt on each
        # of DVE and GpSimd pays a one-time ~9us microcode-load penalty.
        # Fire tiny instances of each during the preamble DMA window.
        vscr = cpool.tile([128, 64], F32, tag="vscr")
        gscr = cpool.tile([128, 64], F32, tag="gscr")
        v.memset(vscr[:], 0.0)
        g.memset(gscr[:], 0.0)
        v.tensor_tensor(vscr[0:4, 0:16], vscr[0:4, 16:32], vscr[0:4, 32:48],
                        ALU.mult)
        v.scalar_tensor_tensor(vscr[0:4, 0:16], vscr[0:4, 16:32], 1.0,
                               vscr[0:4, 32:48], ALU.mult, ALU.add)
        v.tensor_scalar(vscr[0:4, 0:16], vscr[0:4, 16:32], 1.0, None, ALU.mult)
        v.tensor_reduce(vscr[0:4, 0:1], vscr[0:4, 16:32],
                        mybir.AxisListType.X, ALU.add)
        g.tensor_tensor(gscr[0:4, 0:16], gscr[0:4, 16:32], gscr[0:4, 32:48],
                        ALU.mult)
        g.tensor_scalar(gscr[0:4, 0:16], gscr[0:4, 16:32], 1.0, None, ALU.mult)

        if USE_RDMA:
            cid_val = nc.values_load(
                cid_s[0:1, 0:1], engines=[mybir.EngineType.Pool],
                min_val=0, max_val=NCORES - 1, skip_runtime_bounds_check=True)
            bsem = nc.alloc_semaphore("gex_arrive")
            lsem = nc.alloc_semaphore("gex_local")
            psem = nc.alloc_semaphore("gex_prep")

        if (not USE_RDMA) and WARM_CC:
            win = dpool.tile([1, 1], F32, tag="win")
            wout = dpool.tile([1, 1], F32, tag="wout")
            g.dma_start(win[:], ones4[0:1, 0:1])
            g.collective_compute(
                "AllReduce", ALU.add,
                replica_groups=[list(range(NCORES))],
                ins=[win.opt()], outs=[wout.opt()],
            )

        cx = vecs[:, 0:1]
        ct = vecs[:, 1:2]
        cx2 = vecs[:, 2:3]
        cx3 = vecs[:, 3:4]

        def bb(layer):  # bias vector for layer 1..5
            return vecs[:, 3 + layer:4 + layer]

        neg2 = vecs[:, 9:10]

        a5 = ax5 = at5 = axxx5 = None

        with tc.tile_pool(name="ztw", bufs=4, space="PSUM") as zpool:
            # ---------- layer 1 ----------
            # zx/zt/zxx/zxxx for layer1 are per-unit constants cx/ct/etc.
            z = zpool.tile([HB, FD], F32, tag="ztw")
            _mm_chunks(nc, z, w1t[:], h0)
            a = spool.tile([HB, FD], SDT, tag="a")
            s.activation(a[:], z[:], AF.Tanh, bias=bb(1))
            asq = tpool.tile([HB, FD], IDT, tag="asq")
            s.activation(asq[:], a[:], AF.Square)
            h6 = tpool.tile([HB, FD], IDT, tag="h6")
            s.activation(h6[:], asq[:], AF.Identity, scale=6.0, bias=neg2)
            f1 = tpool.tile([HB, FD], IDT, tag="f1")
            s.activation(f1[:], asq[:], AF.Identity, scale=-1.0, bias=1.0)
            ax = spool.tile([HB, FD], SDT, tag="ax")
            v.tensor_scalar(ax[:], f1[:], cx, None, ALU.mult)
            at = spool.tile([HB, FD], SDT, tag="at")
            g.tensor_scalar(at[:], f1[:], ct, None, ALU.mult)
            af1 = tpool.tile([HB, FD], IDT, tag="t2")
            v.tensor_tensor(af1[:], a[:], f1[:], ALU.mult)
            axx = spool.tile([HB, FD], SDT, tag="axx")
            v.tensor_scalar(axx[:], af1[:], cx2, -2.0, ALU.mult, ALU.mult)
            fh = tpool.tile([HB, FD], IDT, tag="t4")
            g.tensor_tensor(fh[:], f1[:], h6[:], ALU.mult)
            axxx = spool.tile([HB, FD], SDT, tag="axxx")
            v.tensor_scalar(axxx[:], fh[:], cx3, None, ALU.mult)

            # ---------- layers 2..5 ----------
            # y=tanh(Wx+b); with f1=1-a^2, h6=6a^2-2:
            #   at   = f1*zt
            #   ax   = f1*zx
            #   axx  = f1*(zxx - 2a*zx^2)
            #   axxx = f1*zxxx + ax*(h6*zx^2 - 6a*zxx)
            for layer in range(2, 6):
                W = wb[:, 100 * (layer - 2):100 * (layer - 1)]
                last = layer == 5

                z = zpool.tile([HB, FD], F32, tag="ztw")
                _mm_chunks(nc, z, W, a)
                zx = zpool.tile([HB, FD], F32, tag="ztw")
                _mm_chunks(nc, zx, W, ax)
                a_n = spool.tile([HB, FD], SDT, tag="a")
                s.activation(a_n[:], z[:], AF.Tanh, bias=bb(layer))
                asq = tpool.tile([HB, FD], IDT, tag="asq")
                s.activation(asq[:], a_n[:], AF.Square)
                w2 = tpool.tile([HB, FD], IDT, tag="w2")
                s.activation(w2[:], zx[:], AF.Square)

                zxx = zpool.tile([HB, FD], F32, tag="ztw")
                _mm_chunks(nc, zxx, W, axx)
                h6 = tpool.tile([HB, FD], IDT, tag="h6")
                s.activation(h6[:], asq[:], AF.Identity, scale=6.0, bias=neg2)
                f1 = tpool.tile([HB, FD], IDT, tag="f1")
                s.activation(f1[:], asq[:], AF.Identity, scale=-1.0, bias=1.0)
                ax_n = spool.tile([HB, FD], SDT, tag="ax")
                v.tensor_tensor(ax_n[:], f1[:], zx[:], ALU.mult)
                t2 = tpool.tile([HB, FD], IDT, tag="t2")
                v.tensor_tensor(t2[:], a_n[:], zxx[:], ALU.mult)
                t4 = tpool.tile([HB, FD], IDT, tag="t4")
                g.tensor_tensor(t4[:], h6[:], w2[:], ALU.mult)
                t5 = tpool.tile([HB, FD], IDT, tag="t5")
                v.scalar_tensor_tensor(t5[:], t2[:], -6.0, t4[:],
                                       ALU.mult, ALU.add)
                t6 = tpool.tile([HB, FD], IDT, tag="t6")
                v.tensor_tensor(t6[:], ax_n[:], t5[:], ALU.mult)

                zxxx = zpool.tile([HB, FD], F32, tag="ztw")
                _mm_chunks(nc, zxxx, W, axxx)
                t1 = tpool.tile([HB, FD], IDT, tag="t1")
                v.tensor_tensor(t1[:], f1[:], zxxx[:], ALU.mult)
                axxx_n = spool.tile([HB, FD], SDT, tag="axxx")
                v.tensor_tensor(axxx_n[:], t1[:], t6[:], ALU.add)

                zt = zpool.tile([HB, FD], F32, tag="ztw")
                _mm_chunks(nc, zt, W, at)
                at_n = spool.tile([HB, FD], SDT, tag="at")
                v.tensor_tensor(at_n[:], f1[:], zt[:], ALU.mult)

                if not last:
                    gt = tpool.tile([HB, FD], IDT, tag="gt")
                    g.tensor_tensor(gt[:], a_n[:], w2[:], ALU.mult)
                    inner = tpool.tile([HB, FD], IDT, tag="inner")
                    v.scalar_tensor_tensor(inner[:], gt[:], -2.0, zxx[:],
                                           ALU.mult, ALU.add)
                    axx_n = spool.tile([HB, FD], SDT, tag="axx")
                    g.tensor_tensor(axx_n[:], f1[:], inner[:], ALU.mult)
                    axx = axx_n

                a, at, ax, axxx = a_n, at_n, ax_n, axxx_n

            a5, ax5, at5, axxx5 = a, ax, at, axxx

        if stage == "tower":
            nc.sync.dma_start(loss_d[:], axxx5[:])
            return

        # ---------- layer 6 + Gram ----------
        # chunk tiles: [128 points, 10] cols: s-major pairs (b0,b1) for
        # s=0 uux, 1 uxxx, 2 ux, 3 ut; cols 8:10 = u.
        with tc.tile_pool(name="l6c", bufs=3, space="PSUM") as l6p, \
             tc.tile_pool(name="psmall", bufs=1, space="PSUM") as pps:
            G = pps.tile([4, 4], F32, tag="gram")
            chvs = []
            for c in range(5):
                lo = 128 * c
                ch = l6p.tile([128, 10], F32, tag="l6c")
                _mm(nc, ch[:, 8:10], a5[:, lo:lo + 128], w6p[:])
                _mm(nc, ch[:, 2:4], axxx5[:, lo:lo + 128], w6p[:])
                _mm(nc, ch[:, 4:6], ax5[:, lo:lo + 128], w6p[:])
                _mm(nc, ch[:, 6:8], at5[:, lo:lo + 128], w6p[:])
                chS = tpool.tile([128, 10], F32, tag=f"l6s{c}")
                s.activation(chS[:, 2:10], ch[:, 2:10], AF.Identity)
                # uux = (u + b6) * ux
                v.scalar_tensor_tensor(chS[:, 0:2], chS[:, 8:10], b6bc[:128, 0:1],
                                       chS[:, 4:6], ALU.add, ALU.mult)
                chv = chS[:, 0:8].rearrange("p (s b) -> p b s", b=2, s=4)
                if c == 4 and B1 < FD:
                    # zero the padded block1 points before the Gram matmul
                    v.tensor_scalar(chv[:, 1, :], chv[:, 1, :], b6bc[:128, 1:2],
                                    None, ALU.mult)
                chvs.append(chv)
            for c in range(5):
                for b in range(2):
                    st = c == 0 and b == 0
                    sp = c == 4 and b == 1
                    nc.tensor.matmul(G[:], chvs[c][:, b, :], chvs[c][:, b, :],
                                     start=st, stop=sp)

            # partial Gram -> SBUF, pre-scaled by 1/N
            s.activation(gS[0:4, :], G[:], AF.Identity, scale=1.0 / NPTS)

            if stage == "l6":
                nc.sync.dma_start(loss_d[:], gS[0:4, :])
                return

            # ---------- exchange partial Grams across the 8 cores ----------
            if USE_RDMA:
                # Each core broadcasts its [128,4] gS to all 8 peers
                # (including itself), into its own column block of recv.
                # All 8 broadcasts -> remote_sem += 2 each -> wait 16.
                for c in range(NCORES):
                    with tc.If(cid_val == c, preferred_fallthrough_block=False):
                        g.remote_dma_broadcast(
                            recv[:, 4 * c:4 * c + 4], gS[:, :],
                            remote_sem=bsem, local_sem=lsem,
                            rdests=[(0, k) for k in range(NCORES)],
                        ).then_inc(psem, 1)
                g.wait_ge(psem, 1)
                if RDMA_BARRIER:
                    g.bir_kernel_barrier_wait([list(range(NCORES))])
                g.trigger_dma(count=1)
                v.wait_ge(bsem, 16)
                Gsum = cpool.tile([4, 4], F32, tag="Gsum")
                rv = recv[0:4, :].rearrange("p (c j) -> p j c", c=NCORES, j=4)
                v.tensor_reduce(Gsum[:], rv, mybir.AxisListType.X, ALU.add)
            else:
                gin = dpool.tile([4, 4], F32, tag="gin")
                gout = dpool.tile([4, 4], F32, tag="gout")
                g.dma_start(gin[:], gS[0:4, :])
                g.collective_compute(
                    "AllReduce", ALU.add,
                    replica_groups=[list(range(NCORES))],
                    ins=[gin.opt()], outs=[gout.opt()],
                )
                Gsum = cpool.tile([4, 4], F32, tag="Gsum")
                g.dma_start(Gsum[:], gout[:])

            if stage == "cc":
                nc.sync.dma_start(loss_d[:], Gsum[:].bitcast(F32))
                return

            # ---------- para quadratic form ----------
            # loss = q4^T G q4 / N  (1/N already folded into G)
            Y = pps.tile([4, PPCP], F32, tag="Y")
            for off, w in PCH:
                _mm(nc, Y[:, off:off + w], Gsum[:].bitcast(F32R),
                    q4[:, off:off + w])
            tq = cpool.tile([4, PPCP], F32R, tag="tq")
            v.tensor_tensor(tq[:], Y[:], q4[:], ALU.mult)
            LP = pps.tile([1, PPCP], F32, tag="LP")
            for off, w in PCH:
                _mm(nc, LP[:, off:off + w], ones4[:].bitcast(F32R),
                    tq[:, off:off + w])
            lossS = cpool.tile([1, PPC], F32, tag="lossS")
            v.tensor_copy(lossS[:], LP[:, 0:PPC])
            nc.sync.dma_start(loss_d[:], lossS[:])


def prep_inputs(x, para, W1, b1, W2, b2, W3, b3, W4, b4, W5, b5, W6, b6):
    """Full inputs -> list of per-core input dicts (host-side shard/layout)."""
    f = np.float32
    x = np.asarray(x, f)
    para = np.asarray(para, f)
    Ws = [np.asarray(W, f) for W in (W1, W2, W3, W4, W5, W6)]
    bs = [np.asarray(b, f) for b in (b1, b2, b3, b4, b5, b6)]

    w1t = np.zeros((4, HB), NDT)
    w1t[0:2, 0:50] = Ws[0].T
    w1t[2:4, 50:100] = Ws[0].T
    wb = np.zeros((HB, 400), NDT)
    for i in range(4):
        W = Ws[i + 1]
        wb[0:50, 100 * i:100 * i + 50] = W.T
        wb[50:100, 100 * i + 50:100 * i + 100] = W.T
    w6p = np.zeros((HB, 2), NDT)
    w6p[0:50, 0] = Ws[5][0]
    w6p[50:100, 1] = Ws[5][0]
    vecs = np.zeros((HB, 10), f)
    vecs[:, 9] = -2.0
    cx = Ws[0][:, 0]
    ct = Ws[0][:, 1]
    for half in (slice(0, 50), slice(50, 100)):
        vecs[half, 0] = cx
        vecs[half, 1] = ct
        vecs[half, 2] = cx * cx
        vecs[half, 3] = cx * cx * cx
        for l in range(5):
            vecs[half, 4 + l] = bs[l]
    b6bc = np.zeros((128, 2), f)
    b6bc[:, 0] = bs[5][0]
    b6bc[:, 1] = 1.0
    b6bc[B1 - 512:, 1] = 0.0

    maps = []
    for c in range(NCORES):
        sl = x[c * NPC:(c + 1) * NPC]
        h0 = np.zeros((4, FD), NDT)
        h0[0] = sl[0:FD, 0]
        h0[1] = sl[0:FD, 1]
        h0[2, 0:B1] = sl[FD:NPC, 0]
        h0[3, 0:B1] = sl[FD:NPC, 1]
        paraT = np.ones((4, PPCP), f)
        paraT[0:3, 0:PPC] = para[c * PPC:(c + 1) * PPC].T
        maps.append({
            "h0": h0, "paraT": paraT, "w1t": w1t, "wb": wb,
            "w6p": w6p, "vecs": vecs, "b6bc": b6bc,
            "cid": np.array([[c]], np.uint32),
        })
    return maps


_NC_CACHE = {}


def get_program():
    if "nc" not in _NC_CACHE:
        _NC_CACHE["nc"] = build_program()
    return _NC_CACHE["nc"]


def kernel(x, para, W1, b1, W2, b2, W3, b3, W4, b4, W5, b5, W6, b6):
    maps = prep_inputs(x, para, W1, b1, W2, b2, W3, b3, W4, b4, W5, b5, W6, b6)
    nc = get_program()
    res = bass_utils.run_bass_kernel_spmd(nc, maps, list(range(NCORES)))
    out = np.concatenate([res.results[c]["loss"].reshape(-1) for c in range(NCORES)])
    return out.astype(np.float32)
